# revision 1
# baseline (speedup 1.0000x reference)
"""Multi-head causal self-attention (torch nn.MultiheadAttention semantics)
on 8 Trainium2 NeuronCores.

Problem: x [2, 2048, 1024], 16 heads, head dim 64, fp32, causal, p_drop=0.

Sharding: 2 batch groups x 4-way head tensor-parallel.
  core c: batch b = c // 4, heads [lane*4, lane*4+4) with lane = c % 4.
Each core computes q/k/v projections for its 4 heads, flash-style causal
attention (S^T score layout, no-max softmax — scores are O(1) here), and its
partial out-projection. The host sums the 4 partials per batch and adds b_out
(this is the all-reduce of the tensor-parallel decomposition, done on host
since the harness contract is full-input -> full-output).

All matmuls run in f32r (reduced-precision fp32 mode of the PE): same
throughput as bf16 (1 cycle/row at moving free >= 256) with ~1.5e-4 matmul
relative error; end-to-end absmax rel err vs the fp32 reference is ~1e-4.

Per-core program details:
  qkT [2*DQ, S] = (wqkT.T @ xT) + bqk  (q and k kept transposed: [dh, seq])
  v' per sk-block: [128, 4*(64+1)] — per-head v with an appended ones column,
     so the PV matmul's row 64 accumulates the softmax denominator for free.
  scores^T block [sk 128, sq 512] = kT.T @ qT; P = exp(s/8) (f32r);
     diagonal blocks multiplied by a precomputed 0/1 causal mask;
  out^T psum [65, 512] accumulates v'.T @ P over sk blocks; row 64 = denom;
     normalized via reciprocal + gpsimd partition-broadcast + DVE mul.
  out [S, DM] partial = OT.T @ woT per 128-row block.
"""

import os
from contextlib import ExitStack
from dataclasses import dataclass

import numpy as np

import concourse.bass as bass
import concourse.tile as tile
from concourse import bacc, mybir
from concourse.bass_utils import run_bass_kernel_spmd

F32 = mybir.dt.float32
F32R = mybir.dt.float32r
AF = mybir.ActivationFunctionType

B = 2
S = 2048
DM = 1024
N_HEADS = 16
DH = 64
N_CORES = 8
CPG = 4  # cores per group (tensor-parallel width over heads)
HPC = N_HEADS // CPG  # heads per core
DQ = HPC * DH
SPAN = 512
SB = 128
NDM = DM // 128
NSPAN = S // SPAN
NSB = S // SB
SBS = SPAN // SB
NQK = 2 * DQ // 128
NHD = DQ // 128
VW = DH + 1
OW = min(512, DM)
NOUT = DM // OW


def _declare_io(nc):
    t = {}
    t["xT"] = nc.dram_tensor("xT", [DM, S], F32R, kind="ExternalInput").ap()
    t["wqkT"] = nc.dram_tensor("wqkT", [DM, 2 * DQ], F32R, kind="ExternalInput").ap()
    t["wvT"] = nc.dram_tensor("wvT", [DM, DQ], F32R, kind="ExternalInput").ap()
    t["woT"] = nc.dram_tensor("woT", [DQ, DM], F32R, kind="ExternalInput").ap()
    t["bqk"] = nc.dram_tensor("bqk", [2 * DQ, 1], F32, kind="ExternalInput").ap()
    t["bv"] = nc.dram_tensor("bv", [128, DQ], F32, kind="ExternalInput").ap()
    t["out"] = nc.dram_tensor("out", [S, DM], F32, kind="ExternalOutput").ap()
    return t


def _build(ctx: ExitStack, tc: tile.TileContext, io: dict):
    nc = tc.nc

    const = ctx.enter_context(tc.tile_pool(name="const", bufs=1))
    work = ctx.enter_context(tc.tile_pool(name="work", bufs=1))
    psum = ctx.enter_context(tc.tile_pool(name="psum", bufs=1, space="PSUM"))

    # ---- constants / inputs ----
    xT = [const.tile([128, S], F32R, name=f"xT{c}") for c in range(NDM)]
    for c in range(NDM):
        nc.sync.dma_start(xT[c][:], io["xT"][c * 128 : (c + 1) * 128, :])

    wqk = [const.tile([128, 2 * DQ], F32R, name=f"wqk{c}") for c in range(NDM)]
    for c in range(NDM):
        nc.sync.dma_start(wqk[c][:], io["wqkT"][c * 128 : (c + 1) * 128, :])

    wv = [const.tile([128, DQ], F32R, name=f"wv{c}") for c in range(NDM)]
    for c in range(NDM):
        nc.sync.dma_start(wv[c][:], io["wvT"][c * 128 : (c + 1) * 128, :])

    wo = [const.tile([128, DM], F32R, name=f"wo{c}") for c in range(NHD)]
    for c in range(NHD):
        nc.sync.dma_start(wo[c][:], io["woT"][c * 128 : (c + 1) * 128, :])

    bqk = [const.tile([128, 1], F32, name=f"bqk{c}") for c in range(NQK)]
    for c in range(NQK):
        nc.sync.dma_start(bqk[c][:], io["bqk"][c * 128 : (c + 1) * 128, :])

    bv = const.tile([128, DQ], F32, name="bv")
    nc.sync.dma_start(bv[:], io["bv"][:])

    # triangular causal mask for the diagonal 128x128 sub-block:
    # tri[r, c] = (c - r >= 0)
    tri = const.tile([128, 128], F32R, name="tri")
    nc.gpsimd.memset(tri[:].bitcast(F32), 1.0)
    nc.gpsimd.affine_select(
        out=tri[:].bitcast(F32),
        in_=tri[:].bitcast(F32),
        compare_op=mybir.AluOpType.is_ge,
        fill=0.0,
        base=0,
        pattern=[[1, 128]],
        channel_multiplier=-1,
    )

    # ---- phase 1: q/k projection (transposed layout) ----
    qkT = [const.tile([128, S], F32R, name=f"qkT{b}") for b in range(NQK)]
    for ob in range(NQK):
        for sp in range(NSPAN):
            pqk = psum.tile([128, SPAN], F32, name=f"pqk_{ob}_{sp}", tag="po", bufs=4)
            for c in range(NDM):
                nc.tensor.matmul(
                    pqk[:],
                    wqk[c][:, ob * 128 : (ob + 1) * 128],
                    xT[c][:, sp * SPAN : (sp + 1) * SPAN],
                    start=(c == 0),
                    stop=(c == NDM - 1),
                )
            nc.vector.tensor_scalar_add(
                qkT[ob][:, sp * SPAN : (sp + 1) * SPAN], pqk[:], bqk[ob][:]
            )

    # ---- phase 2: v projection into v' (per-head + ones column) ----
    vp = [const.tile([128, HPC * VW], F32R, name=f"vp{sb}") for sb in range(NSB)]
    for sb in range(NSB):
        pv = psum.tile([128, DQ], F32, name=f"pv_{sb}", tag="po", bufs=4)
        for c in range(NDM):
            nc.tensor.matmul(
                pv[:],
                xT[c][:, sb * 128 : (sb + 1) * 128],
                wv[c][:],
                start=(c == 0),
                stop=(c == NDM - 1),
            )
        vdst = vp[sb][:, 0 : HPC * VW].rearrange("p (h w) -> p h w", w=VW)[:, :, 0:DH]
        nc.vector.tensor_add(
            vdst,
            pv[:].rearrange("p (h d) -> p h d", d=DH),
            bv[:].rearrange("p (h d) -> p h d", d=DH),
        )
        ones_cols = vp[sb][:, DH : HPC * VW : VW]
        nc.vector.memset(ones_cols.bitcast(F32), 1.0)

    # ---- phase 3+4: attention (flash, S^T layout) + interleaved out-proj ----
    # Per sk-block group: all HPC heads' score matmuls (uniform K=64 shape),
    # then all HPC heads' PV matmuls (uniform K=128 shape, distinct PSUM
    # banks), PVs lagging one group so the exp chain stays off PE's critical
    # path. Shape-uniform runs keep the PE array from draining between
    # matmuls (alternating K=64/K=128 measured 672 ns/mm vs 232 uniform).
    OT = [const.tile([128, S], F32R, name=f"OT{c}") for c in range(NHD)]
    for sp in range(NSPAN):
        den = work.tile([32 * (HPC - 1) + 1, SPAN], F32, name=f"den_{sp}", tag="den", bufs=1)
        nsb = (sp + 1) * SBS  # causal: sk blocks up to the span end
        pos = {}
        pts = {}
        oraw = {}

        def emit_scores(sb):
            for h in range(HPC):
                qt = qkT[h // 2]
                kt = qkT[NQK // 2 + h // 2]
                qrow = (h % 2) * 64
                ps = psum.tile(
                    [128, SPAN], F32, name=f"ps_{h}_{sp}_{sb}", tag="ps", bufs=4
                )
                nc.tensor.matmul(
                    ps[:],
                    kt[qrow : qrow + 64, sb * 128 : (sb + 1) * 128],
                    qt[qrow : qrow + 64, sp * SPAN : (sp + 1) * SPAN],
                    start=True,
                    stop=True,
                )
                pt = work.tile(
                    [128, SPAN], F32R, name=f"pt_{h}_{sp}_{sb}", tag="pt", bufs=8
                )
                pts[(h, sb)] = pt
                d = sb - sp * SBS
                if d < 0:
                    nc.scalar.activation(pt[:], ps[:], AF.Exp, scale=0.125)
                else:
                    # diagonal block: cols < 128*d fully masked, then one
                    # triangular 128x128 sub-block
                    if d > 0:
                        nc.vector.memset(pt[:, 0 : 128 * d].bitcast(F32), 0.0)
                    nc.scalar.activation(
                        pt[:, 128 * d : SPAN], ps[:, 128 * d : SPAN],
                        AF.Exp, scale=0.125,
                    )
                    nc.vector.tensor_mul(
                        pt[:, 128 * d : 128 * (d + 1)],
                        pt[:, 128 * d : 128 * (d + 1)],
                        tri[:],
                    )

        def emit_pvs(sb):
            for h in range(HPC):
                if sb == 0:
                    pos[h] = psum.tile(
                        [VW, SPAN], F32, name=f"po_{h}_{sp}", tag="po", bufs=4
                    )
                nc.tensor.matmul(
                    pos[h][:],
                    vp[sb][:, h * VW : (h + 1) * VW],
                    pts.pop((h, sb))[:],
                    start=(sb == 0),
                    stop=(sb == nsb - 1),
                )
                if sb == nsb - 1:
                    # copy (out^T | denom) to SBUF to free the PSUM bank early
                    orw = work.tile(
                        [VW, SPAN], F32, name=f"oraw_{h}_{sp}", tag="oraw", bufs=4
                    )
                    oraw[h] = orw
                    nc.vector.tensor_copy(orw[:], pos[h][:])
                    nc.vector.tensor_copy(den[32 * h : 32 * h + 1, :], orw[VW - 1 : VW, :])

        for i in range(nsb + 1):
            if i < nsb:
                emit_scores(i)
            if i >= 1:
                emit_pvs(i - 1)

        denr = work.tile([32 * (HPC - 1) + 1, SPAN], F32, name=f"denr_{sp}", tag="denr", bufs=1)
        # only rows 0/32/64/96 are meaningful; reciprocal of the garbage
        # rows in between is never read
        nc.vector.reciprocal(denr[:], den[:])
        for h in range(HPC):
            ot_tile = OT[(h * DH) // 128]
            orow = (h * DH) % 128
            rtmp = work.tile([1, SPAN], F32, name=f"rtmp_{h}_{sp}", tag="rtmp", bufs=2)
            # partition_broadcast needs a partition-0 source
            nc.vector.tensor_copy(rtmp[:], denr[32 * h : 32 * h + 1, :])
            recb = work.tile([DH, SPAN], F32, name=f"recb_{h}_{sp}", tag="recb", bufs=2)
            nc.gpsimd.partition_broadcast(recb[:], rtmp[0:1, :])
            nc.vector.tensor_mul(
                ot_tile[orow : orow + DH, sp * SPAN : (sp + 1) * SPAN],
                oraw[h][0:DH, :],
                recb[:],
            )
    # out projection for this span's sq blocks
        for qb in range(sp * SBS, (sp + 1) * SBS):
            ob = work.tile([128, DM], F32, name=f"ob_{qb}", tag="ob", bufs=2)
            for nh in range(NOUT):
                pot = psum.tile([128, OW], F32, name=f"pot_{qb}_{nh}", tag="po", bufs=4)
                for c in range(NHD):
                    nc.tensor.matmul(
                        pot[:],
                        OT[c][:, qb * 128 : (qb + 1) * 128],
                        wo[c][:, nh * OW : (nh + 1) * OW],
                        start=(c == 0),
                        stop=(c == NHD - 1),
                    )
                if (qb + nh) % 2 == 0:
                    nc.scalar.copy(ob[:, nh * OW : (nh + 1) * OW], pot[:])
                else:
                    nc.vector.tensor_copy(ob[:, nh * OW : (nh + 1) * OW], pot[:])
            nc.sync.dma_start(io["out"][qb * 128 : (qb + 1) * 128, :], ob[:])


_NC_CACHE = {}


def _get_compiled():
    if "nc" not in _NC_CACHE:
        nc = bacc.Bacc(
            "TRN2", target_bir_lowering=False, debug=False, num_devices=N_CORES
        )
        io = _declare_io(nc)
        with tile.TileContext(nc) as tc, ExitStack() as ctx:
            _build(ctx, tc, io)
        nc.compile()
        _NC_CACHE["nc"] = nc
    return _NC_CACHE["nc"]


def _prep_core_inputs(x, W_qkv, b_qkv, W_out, b_out, core_id):
    g = core_id // CPG
    lane = core_id % CPG
    h0 = lane * HPC
    r = slice(h0 * DH, (h0 + HPC) * DH)
    Wq = W_qkv[0 * DM : 1 * DM, :][r, :]
    Wk = W_qkv[1 * DM : 2 * DM, :][r, :]
    Wv = W_qkv[2 * DM : 3 * DM, :][r, :]
    bq = b_qkv[0 * DM + h0 * DH : 0 * DM + (h0 + HPC) * DH]
    bk = b_qkv[1 * DM + h0 * DH : 1 * DM + (h0 + HPC) * DH]
    bv_ = b_qkv[2 * DM + h0 * DH : 2 * DM + (h0 + HPC) * DH]
    return {
        "xT": np.ascontiguousarray(x[g].T.astype(np.float32)),
        "wqkT": np.ascontiguousarray(
            np.concatenate([Wq.T, Wk.T], axis=1).astype(np.float32)
        ),
        "wvT": np.ascontiguousarray(Wv.T.astype(np.float32)),
        "woT": np.ascontiguousarray(W_out[:, r].T.astype(np.float32)),
        "bqk": np.concatenate([bq, bk]).reshape(2 * DQ, 1).astype(np.float32),
        "bv": np.ascontiguousarray(
            np.broadcast_to(bv_.reshape(1, DQ), (128, DQ)).astype(np.float32)
        ),
    }


def kernel(x, W_qkv, b_qkv, W_out, b_out, _trace=False):
    x = np.asarray(x)
    W_qkv = np.asarray(W_qkv)
    b_qkv = np.asarray(b_qkv)
    W_out = np.asarray(W_out)
    b_out = np.asarray(b_out)

    nc = _get_compiled()
    in_maps = [
        _prep_core_inputs(x, W_qkv, b_qkv, W_out, b_out, c) for c in range(N_CORES)
    ]
    res = run_bass_kernel_spmd(nc, in_maps, list(range(N_CORES)), trace=_trace)

    out = np.empty((B, S, DM), dtype=np.float32)
    for g in range(B):
        acc = res.results[g * CPG]["out"].astype(np.float32)
        for lane in range(1, CPG):
            acc = acc + res.results[g * CPG + lane]["out"]
        out[g] = acc + b_out[None, :].astype(np.float32)

    if _trace:
        kernel.last_exec_time_ns = res.exec_time_ns
        kernel.last_results = res
    return out



# revision 2
# speedup vs baseline: 1.3821x; 1.3821x over previous
"""Multi-head causal self-attention (torch nn.MultiheadAttention semantics)
on 8 Trainium2 NeuronCores.

Problem: x [2, 2048, 1024], 16 heads, head dim 64, fp32, causal, p_drop=0.

Sharding: 2 batch groups x 4-way head tensor-parallel.
  core c: batch b = c // 4, heads [lane*4, lane*4+4) with lane = c % 4.
Each core computes q/k/v projections for its 4 heads, flash-style causal
attention (S^T score layout, no-max softmax — scores are O(1) here), and its
partial out-projection. The host sums the 4 partials per batch and adds b_out
(the all-reduce of the tensor-parallel decomposition, free on host since the
harness contract is full-input -> full-output).

All matmuls run in bf16 (1 cycle/row on the PE, FWL weight loads) with fp32
PSUM accumulation; rel err vs the fp32 reference lands ~1e-3 against the
2e-2 gate. Program order interleaves per span so compute starts ~2MB into
the input DMA instead of after all of it:
  for span: qk-proj -> v-proj -> attention -> out-proj.

Attention per span, per sk-block: score matmuls for a head PAIR are emitted
back-to-back with lhsT base partitions 0/64 -> auto tile_position (0,0) /
(64,0) -> the two K=64 matmuls run concurrently in disjoint PE row-groups.
Each pair writes one [128, 1024] 2-bank PSUM tile; ONE merged ACTIVATE exps
both heads' scores (amortizes the ~293ns ACT instruction overhead, the
attention-phase bottleneck). Diagonal blocks memset the fully-masked prefix
and exp only the live region via a strided per-head view, then multiply the
diagonal 128x128 sub-block by a host-provided 0/1 triangular mask.

v' per sk-block: [128, 4*(64+1)] — per-head v with an appended ones column,
so the PV matmul's row 64 accumulates the softmax denominator for free.
out^T psum [65, 512] accumulates v'.T @ P over sk blocks; normalized via
reciprocal_approx_fast + gpsimd partition-broadcast + DVE mul.
out [S, DM] partial = OT.T @ woT per 128-row block, written back as bf16;
the host sums the 4 bf16 partials per batch in fp32 and adds b_out.
"""

from contextlib import ExitStack

import numpy as np
import ml_dtypes

import concourse.bass as bass
import concourse.tile as tile
from concourse import bacc, mybir
from concourse.bass_utils import run_bass_kernel_spmd

F32 = mybir.dt.float32
BF16 = mybir.dt.bfloat16
AF = mybir.ActivationFunctionType

B = 2
S = 2048
DM = 1024
N_HEADS = 16
DH = 64
N_CORES = 8
CPG = 4  # cores per group (tensor-parallel width over heads)
HPC = N_HEADS // CPG  # heads per core
DQ = HPC * DH
SPAN = 512
SB = 128
NDM = DM // 128
NSPAN = S // SPAN
NSB = S // SB
SBS = SPAN // SB
NQK = 2 * DQ // 128
NHD = DQ // 128
VW = DH + 1
OW = min(512, DM)
NOUT = DM // OW
NPAIR = HPC // 2


def _declare_io(nc):
    t = {}
    t["xT"] = nc.dram_tensor("xT", [DM, S], BF16, kind="ExternalInput").ap()
    t["wqkT"] = nc.dram_tensor("wqkT", [DM, 2 * DQ], BF16, kind="ExternalInput").ap()
    t["wvT"] = nc.dram_tensor("wvT", [DM, DQ], BF16, kind="ExternalInput").ap()
    t["woT"] = nc.dram_tensor("woT", [DQ, DM], BF16, kind="ExternalInput").ap()
    t["bqk"] = nc.dram_tensor("bqk", [2 * DQ, 1], F32, kind="ExternalInput").ap()
    t["bv"] = nc.dram_tensor("bv", [128, DQ], F32, kind="ExternalInput").ap()
    t["tri"] = nc.dram_tensor("tri", [128, 128], BF16, kind="ExternalInput").ap()
    t["out"] = nc.dram_tensor("out", [S, DM], BF16, kind="ExternalOutput").ap()
    return t


def _build(ctx: ExitStack, tc: tile.TileContext, io: dict):
    nc = tc.nc

    const = ctx.enter_context(tc.tile_pool(name="const", bufs=1))
    work = ctx.enter_context(tc.tile_pool(name="work", bufs=1))
    psum = ctx.enter_context(tc.tile_pool(name="psum", bufs=1, space="PSUM"))

    # ---- weights first (small), then x span by span so phase 1 of span 0
    # can start ~2MB into the DMA stream ----
    wqk = [const.tile([128, 2 * DQ], BF16, name=f"wqk{c}") for c in range(NDM)]
    for c in range(NDM):
        nc.sync.dma_start(wqk[c][:], io["wqkT"][c * 128 : (c + 1) * 128, :])
    wv = [const.tile([128, DQ], BF16, name=f"wv{c}") for c in range(NDM)]
    for c in range(NDM):
        nc.sync.dma_start(wv[c][:], io["wvT"][c * 128 : (c + 1) * 128, :])

    bqk = [const.tile([128, 1], F32, name=f"bqk{c}") for c in range(NQK)]
    for c in range(NQK):
        nc.sync.dma_start(bqk[c][:], io["bqk"][c * 128 : (c + 1) * 128, :])
    bv = const.tile([128, DQ], F32, name="bv")
    nc.sync.dma_start(bv[:], io["bv"][:])
    tri = const.tile([128, 128], BF16, name="tri")
    nc.sync.dma_start(tri[:], io["tri"][:])

    # x, span-major: all dm-chunks of span 0, then span 1, ...
    xT = [[None] * NSPAN for _ in range(NDM)]
    for sp in range(NSPAN):
        for c in range(NDM):
            xT[c][sp] = const.tile([128, SPAN], BF16, name=f"xT{c}_{sp}")
            nc.sync.dma_start(
                xT[c][sp][:],
                io["xT"][c * 128 : (c + 1) * 128, sp * SPAN : (sp + 1) * SPAN],
            )

    wo = [const.tile([128, DM], BF16, name=f"wo{c}") for c in range(NHD)]
    for c in range(NHD):
        nc.sync.dma_start(wo[c][:], io["woT"][c * 128 : (c + 1) * 128, :])

    qkT = [const.tile([128, S], BF16, name=f"qkT{b}") for b in range(NQK)]
    vp = [const.tile([128, HPC * VW], BF16, name=f"vp{sb}") for sb in range(NSB)]
    OT = [const.tile([128, S], BF16, name=f"OT{c}") for c in range(NHD)]

    for sp in range(NSPAN):
        # ---- qk projection for this span (transposed layout) ----
        for ob in range(NQK):
            pqk = psum.tile([128, SPAN], F32, name=f"pqk_{ob}_{sp}", tag="po", bufs=4)
            for c in range(NDM):
                nc.tensor.matmul(
                    pqk[:],
                    wqk[c][:, ob * 128 : (ob + 1) * 128],
                    xT[c][sp][:],
                    start=(c == 0),
                    stop=(c == NDM - 1),
                )
            nc.vector.tensor_scalar_add(
                qkT[ob][:, sp * SPAN : (sp + 1) * SPAN], pqk[:], bqk[ob][:]
            )

        # ---- v projection for this span's sk blocks (v' layout) ----
        for j in range(SBS):
            sb = sp * SBS + j
            pv = psum.tile([128, DQ], F32, name=f"pv_{sb}", tag="po", bufs=4)
            for c in range(NDM):
                nc.tensor.matmul(
                    pv[:],
                    xT[c][sp][:, j * 128 : (j + 1) * 128],
                    wv[c][:],
                    start=(c == 0),
                    stop=(c == NDM - 1),
                )
            vdst = vp[sb][:, 0 : HPC * VW].rearrange("p (h w) -> p h w", w=VW)[
                :, :, 0:DH
            ]
            nc.vector.tensor_add(
                vdst,
                pv[:].rearrange("p (h d) -> p h d", d=DH),
                bv[:].rearrange("p (h d) -> p h d", d=DH),
            )
            ones_cols = vp[sb][:, DH : HPC * VW : VW]
            nc.vector.memset(ones_cols, 1.0)

        # ---- attention for this span (flash, S^T layout, head pairs) ----
        den = work.tile(
            [32 * (HPC - 1) + 1, SPAN], F32, name=f"den_{sp}", tag="den", bufs=1
        )
        nsb = (sp + 1) * SBS  # causal: sk blocks up to the span end
        pos = {}
        pts = {}
        oraw = {}

        def emit_scores(sb):
            for p in range(NPAIR):
                qt = qkT[p]
                kt = qkT[NQK // 2 + p]
                ps = psum.tile(
                    [128, 2 * SPAN], F32, name=f"ps_{p}_{sp}_{sb}", tag="ps", bufs=2
                )
                for i in range(2):
                    r = i * 64
                    nc.tensor.matmul(
                        ps[:, i * SPAN : (i + 1) * SPAN],
                        kt[r : r + 64, sb * 128 : (sb + 1) * 128],
                        qt[r : r + 64, sp * SPAN : (sp + 1) * SPAN],
                        start=True,
                        stop=True,
                    )
                pt = work.tile(
                    [128, 2 * SPAN], BF16, name=f"pt_{p}_{sp}_{sb}", tag="pt", bufs=4
                )
                pts[(p, sb)] = pt
                d = sb - sp * SBS
                if d < 0:
                    nc.scalar.activation(pt[:], ps[:], AF.Exp, scale=0.125)
                else:
                    # diagonal block: per head, cols < 128*d fully masked,
                    # then one triangular 128x128 sub-block
                    ptv = pt[:].rearrange("p (h w) -> p h w", w=SPAN)
                    psv = ps[:].rearrange("p (h w) -> p h w", w=SPAN)
                    if d > 0:
                        nc.vector.memset(ptv[:, :, 0 : 128 * d], 0.0)
                    nc.scalar.activation(
                        ptv[:, :, 128 * d : SPAN],
                        psv[:, :, 128 * d : SPAN],
                        AF.Exp,
                        scale=0.125,
                    )
                    for i in range(2):
                        nc.vector.tensor_mul(
                            pt[:, i * SPAN + 128 * d : i * SPAN + 128 * (d + 1)],
                            pt[:, i * SPAN + 128 * d : i * SPAN + 128 * (d + 1)],
                            tri[:],
                        )

        def emit_pvs(sb):
            for h in range(HPC):
                if sb == 0:
                    pos[h] = psum.tile(
                        [VW, SPAN], F32, name=f"po_{h}_{sp}", tag="po", bufs=4
                    )
                pt = pts[(h // 2, sb)]
                nc.tensor.matmul(
                    pos[h][:],
                    vp[sb][:, h * VW : (h + 1) * VW],
                    pt[:, (h % 2) * SPAN : (h % 2 + 1) * SPAN],
                    start=(sb == 0),
                    stop=(sb == nsb - 1),
                )
                if h % 2 == 1:
                    pts.pop((h // 2, sb))
                if sb == nsb - 1:
                    # copy (out^T | denom) to SBUF to free the PSUM bank early
                    orw = work.tile(
                        [VW, SPAN], F32, name=f"oraw_{h}_{sp}", tag="oraw", bufs=4
                    )
                    oraw[h] = orw
                    nc.vector.tensor_copy(orw[:], pos[h][:])
                    nc.vector.tensor_copy(
                        den[32 * h : 32 * h + 1, :], orw[VW - 1 : VW, :]
                    )

        for i in range(nsb + 1):
            if i < nsb:
                emit_scores(i)
            if i >= 1:
                emit_pvs(i - 1)

        denr = work.tile(
            [32 * (HPC - 1) + 1, SPAN], F32, name=f"denr_{sp}", tag="denr", bufs=1
        )
        # only rows 0/32/64/96 are meaningful; recip of the garbage rows in
        # between is never read. ~51 ULP accuracy is plenty for a softmax
        # denominator against the 2e-2 gate.
        nc.vector.reciprocal_approx_fast(denr[:], den[:])
        for h in range(HPC):
            ot_tile = OT[(h * DH) // 128]
            orow = (h * DH) % 128
            rtmp = work.tile([1, SPAN], F32, name=f"rtmp_{h}_{sp}", tag="rtmp", bufs=2)
            # partition_broadcast needs a partition-0 source
            nc.vector.tensor_copy(rtmp[:], denr[32 * h : 32 * h + 1, :])
            recb = work.tile([DH, SPAN], F32, name=f"recb_{h}_{sp}", tag="recb", bufs=2)
            nc.gpsimd.partition_broadcast(recb[:], rtmp[0:1, :])
            nc.vector.tensor_mul(
                ot_tile[orow : orow + DH, sp * SPAN : (sp + 1) * SPAN],
                oraw[h][0:DH, :],
                recb[:],
            )

        # ---- out projection for this span's sq blocks ----
        for qb in range(sp * SBS, (sp + 1) * SBS):
            ob = work.tile([128, DM], BF16, name=f"ob_{qb}", tag="ob", bufs=2)
            for nh in range(NOUT):
                pot = psum.tile([128, OW], F32, name=f"pot_{qb}_{nh}", tag="po", bufs=4)
                for c in range(NHD):
                    nc.tensor.matmul(
                        pot[:],
                        OT[c][:, qb * 128 : (qb + 1) * 128],
                        wo[c][:, nh * OW : (nh + 1) * OW],
                        start=(c == 0),
                        stop=(c == NHD - 1),
                    )
                if (qb + nh) % 2 == 0:
                    nc.scalar.copy(ob[:, nh * OW : (nh + 1) * OW], pot[:])
                else:
                    nc.vector.tensor_copy(ob[:, nh * OW : (nh + 1) * OW], pot[:])
            nc.sync.dma_start(io["out"][qb * 128 : (qb + 1) * 128, :], ob[:])


_NC_CACHE = {}


def _get_compiled():
    if "nc" not in _NC_CACHE:
        nc = bacc.Bacc(
            "TRN2", target_bir_lowering=False, debug=False, num_devices=N_CORES
        )
        io = _declare_io(nc)
        with tile.TileContext(nc) as tc, ExitStack() as ctx:
            _build(ctx, tc, io)
        nc.compile()
        _NC_CACHE["nc"] = nc
    return _NC_CACHE["nc"]


def _prep_core_inputs(x, W_qkv, b_qkv, W_out, b_out, core_id, tri):
    bf16 = ml_dtypes.bfloat16
    g = core_id // CPG
    lane = core_id % CPG
    h0 = lane * HPC
    r = slice(h0 * DH, (h0 + HPC) * DH)
    Wq = W_qkv[0 * DM : 1 * DM, :][r, :]
    Wk = W_qkv[1 * DM : 2 * DM, :][r, :]
    Wv = W_qkv[2 * DM : 3 * DM, :][r, :]
    bq = b_qkv[0 * DM + h0 * DH : 0 * DM + (h0 + HPC) * DH]
    bk = b_qkv[1 * DM + h0 * DH : 1 * DM + (h0 + HPC) * DH]
    bv_ = b_qkv[2 * DM + h0 * DH : 2 * DM + (h0 + HPC) * DH]
    return {
        "xT": np.ascontiguousarray(x[g].T.astype(bf16)),
        "wqkT": np.ascontiguousarray(
            np.concatenate([Wq.T, Wk.T], axis=1).astype(bf16)
        ),
        "wvT": np.ascontiguousarray(Wv.T.astype(bf16)),
        "woT": np.ascontiguousarray(W_out[:, r].T.astype(bf16)),
        "bqk": np.concatenate([bq, bk]).reshape(2 * DQ, 1).astype(np.float32),
        "bv": np.ascontiguousarray(
            np.broadcast_to(bv_.reshape(1, DQ), (128, DQ)).astype(np.float32)
        ),
        "tri": tri,
    }


def kernel(x, W_qkv, b_qkv, W_out, b_out, _trace=False):
    x = np.asarray(x)
    W_qkv = np.asarray(W_qkv)
    b_qkv = np.asarray(b_qkv)
    W_out = np.asarray(W_out)
    b_out = np.asarray(b_out)

    # tri[r, c] = (c >= r): keep (k, q) where q >= k in the diagonal block
    tri = np.triu(np.ones((128, 128), dtype=np.float32)).astype(ml_dtypes.bfloat16)

    nc = _get_compiled()
    in_maps = [
        _prep_core_inputs(x, W_qkv, b_qkv, W_out, b_out, c, tri)
        for c in range(N_CORES)
    ]
    res = run_bass_kernel_spmd(nc, in_maps, list(range(N_CORES)), trace=_trace)

    out = np.empty((B, S, DM), dtype=np.float32)
    for g in range(B):
        acc = res.results[g * CPG]["out"].astype(np.float32)
        for lane in range(1, CPG):
            acc = acc + res.results[g * CPG + lane]["out"].astype(np.float32)
        out[g] = acc + b_out[None, :].astype(np.float32)

    if _trace:
        kernel.last_exec_time_ns = res.exec_time_ns
        kernel.last_results = res
    return out


# revision 8
# speedup vs baseline: 1.5168x; 1.0974x over previous
"""Multi-head causal self-attention (torch nn.MultiheadAttention semantics)
on 8 Trainium2 NeuronCores.

Problem: x [2, 2048, 1024], 16 heads, head dim 64, fp32, causal, p_drop=0.

Sharding: 2 batch groups x 4-way head tensor-parallel.
  core c: batch b = c // 4, heads [lane*4, lane*4+4) with lane = c % 4.
Each core computes q/k/v projections for its 4 heads, flash-style causal
attention (S^T score layout, no-max softmax — scores are O(1) here), and its
partial out-projection. The host sums the 4 partials per batch and adds b_out
(the all-reduce of the tensor-parallel decomposition, free on host since the
harness contract is full-input -> full-output).

All matmuls run in bf16 (1 cycle/row on the PE, FWL weight loads) with fp32
PSUM accumulation; rel err vs the fp32 reference lands ~4e-3 against the
2e-2 gate.

Engine balance: the attention inner loop is ACT-bound (exp at 1 elem/cycle/
lane + ~293ns/instruction overhead), while the projections are PE-bound. So
the q/k/v projection of span sp+1 and the out-projection of span sp-1 are
emitted as small matmul groups INTERLEAVED into span sp's attention block
loop — the PE fills its exp-wait gaps with projection work and never idles
>3.4us (which would re-throttle the HAM clock gate to 1.2 GHz).

Attention per span, per sk-block: score matmuls for a head PAIR are emitted
back-to-back with lhsT base partitions 0/64 -> auto tile_position (0,0) /
(64,0) -> the two K=64 matmuls run concurrently in disjoint PE row-groups
(measured dstart ~4ns). Each pair writes one [128, 1024] 2-bank PSUM tile;
ONE merged ACTIVATE exps both heads' scores. Diagonal blocks memset the
fully-masked prefix and exp only the live region via a strided per-head
view, then multiply the diagonal 128x128 sub-block by a host-provided 0/1
triangular mask.

v' per sk-block: [128, 4*(64+1)] — per-head v with an appended ones column,
so the PV matmul's row 64 accumulates the softmax denominator for free.
out^T psum [65, 512] accumulates v'.T @ P over sk blocks; row 64 = denom;
normalized via reciprocal_approx_fast + gpsimd partition-broadcast + DVE
mul. out [S, DM] partial = OT.T @ woT per 128-row block, written back as
bf16; the host sums the 4 bf16 partials per batch in fp32 and adds b_out.

PSUM (8 banks): ps pair-score tiles [128,1024] x1 buf = 2 banks; po PV
accumulators [65,512] x4 = 4 banks; pp projection tiles [128,512] x2 = 2.
"""

from contextlib import ExitStack

import numpy as np
import ml_dtypes

import concourse.bass as bass
import concourse.tile as tile
from concourse import bacc, mybir
from concourse.bass_utils import run_bass_kernel_spmd

F32 = mybir.dt.float32
BF16 = mybir.dt.bfloat16
AF = mybir.ActivationFunctionType

B = 2
S = 2048
DM = 1024
N_HEADS = 16
DH = 64
N_CORES = 8
CPG = 4  # cores per group (tensor-parallel width over heads)
HPC = N_HEADS // CPG  # heads per core
DQ = HPC * DH
SPAN = 512
SB = 128
NDM = DM // 128
NSPAN = S // SPAN
NSB = S // SB
SBS = SPAN // SB
NQK = 2 * DQ // 128
NHD = DQ // 128
VW = DH + 1
OW = min(512, DM)
NOUT = DM // OW
NPAIR = HPC // 2


def _declare_io(nc):
    t = {}
    t["xT"] = nc.dram_tensor("xT", [DM, S], BF16, kind="ExternalInput").ap()
    t["wqkT"] = nc.dram_tensor("wqkT", [DM, 2 * DQ], BF16, kind="ExternalInput").ap()
    t["wvT"] = nc.dram_tensor("wvT", [DM, DQ], BF16, kind="ExternalInput").ap()
    t["woT"] = nc.dram_tensor("woT", [DQ, DM], BF16, kind="ExternalInput").ap()
    t["bqk"] = nc.dram_tensor("bqk", [2 * DQ, 1], F32, kind="ExternalInput").ap()
    t["bv"] = nc.dram_tensor("bv", [128, DQ], F32, kind="ExternalInput").ap()
    t["tri"] = nc.dram_tensor("tri", [128, 128], BF16, kind="ExternalInput").ap()
    t["out"] = nc.dram_tensor("out", [S, DM], BF16, kind="ExternalOutput").ap()
    return t


def _build(ctx: ExitStack, tc: tile.TileContext, io: dict):
    nc = tc.nc

    const = ctx.enter_context(tc.tile_pool(name="const", bufs=1))
    work = ctx.enter_context(tc.tile_pool(name="work", bufs=1))
    psum = ctx.enter_context(tc.tile_pool(name="psum", bufs=1, space="PSUM"))

    # ---- DMA order: minimum prefix for qk-proj of span 0 (wqk + xT sp0)
    # first, then the rest roughly in first-use order ----
    wqk = [const.tile([128, 2 * DQ], BF16, name=f"wqk{c}") for c in range(NDM)]
    for c in range(NDM):
        nc.sync.dma_start(wqk[c][:], io["wqkT"][c * 128 : (c + 1) * 128, :])

    xT = [[None] * NSPAN for _ in range(NDM)]

    def dma_x_span(sp):
        for c in range(NDM):
            xT[c][sp] = const.tile([128, SPAN], BF16, name=f"xT{c}_{sp}")
            nc.sync.dma_start(
                xT[c][sp][:],
                io["xT"][c * 128 : (c + 1) * 128, sp * SPAN : (sp + 1) * SPAN],
            )

    dma_x_span(0)
    wv = [const.tile([128, DQ], BF16, name=f"wv{c}") for c in range(NDM)]
    for c in range(NDM):
        nc.sync.dma_start(wv[c][:], io["wvT"][c * 128 : (c + 1) * 128, :])

    bqk = [const.tile([128, 1], F32, name=f"bqk{c}") for c in range(NQK)]
    for c in range(NQK):
        nc.sync.dma_start(bqk[c][:], io["bqk"][c * 128 : (c + 1) * 128, :])
    bv = const.tile([128, DQ], F32, name="bv")
    nc.sync.dma_start(bv[:], io["bv"][:])
    tri = const.tile([128, 128], BF16, name="tri")
    nc.sync.dma_start(tri[:], io["tri"][:])

    dma_x_span(1)
    wo = [const.tile([128, DM], BF16, name=f"wo{c}") for c in range(NHD)]
    for c in range(NHD):
        nc.sync.dma_start(wo[c][:], io["woT"][c * 128 : (c + 1) * 128, :])
    dma_x_span(2)
    dma_x_span(3)

    qkT = [const.tile([128, S], BF16, name=f"qkT{b}") for b in range(NQK)]
    vp = [const.tile([128, HPC * VW], BF16, name=f"vp{sb}") for sb in range(NSB)]
    OT = [const.tile([128, S], BF16, name=f"OT{c}") for c in range(NHD)]

    # ---- projection / out-projection emitters, one small PE group each,
    # suitable for interleaving into the attention block loop ----
    def pqk_group(sp, ob):
        pqk = psum.tile([128, SPAN], F32, name=f"pqk_{ob}_{sp}", tag="pp", bufs=2)
        for c in range(NDM):
            nc.tensor.matmul(
                pqk[:],
                wqk[c][:, ob * 128 : (ob + 1) * 128],
                xT[c][sp][:],
                start=(c == 0),
                stop=(c == NDM - 1),
            )
        nc.vector.tensor_scalar_add(
            qkT[ob][:, sp * SPAN : (sp + 1) * SPAN], pqk[:], bqk[ob][:]
        )

    def pv_group(sp, j):
        sb = sp * SBS + j
        pv = psum.tile([128, DQ], F32, name=f"pv_{sb}", tag="pp", bufs=2)
        for c in range(NDM):
            nc.tensor.matmul(
                pv[:],
                xT[c][sp][:, j * 128 : (j + 1) * 128],
                wv[c][:],
                start=(c == 0),
                stop=(c == NDM - 1),
            )
        vdst = vp[sb][:, 0 : HPC * VW].rearrange("p (h w) -> p h w", w=VW)[:, :, 0:DH]
        nc.vector.tensor_add(
            vdst,
            pv[:].rearrange("p (h d) -> p h d", d=DH),
            bv[:].rearrange("p (h d) -> p h d", d=DH),
        )
        ones_cols = vp[sb][:, DH : HPC * VW : VW]
        nc.vector.memset(ones_cols, 1.0)

    ob_tiles = {}

    def pot_group(sp, qb, nh, tail=False):
        if nh == 0:
            ob_tiles[qb] = work.tile([128, DM], BF16, name=f"ob_{qb}", tag="ob", bufs=2)
        ob = ob_tiles[qb]
        pot = psum.tile([128, OW], F32, name=f"pot_{qb}_{nh}", tag="pp", bufs=2)
        for c in range(NHD):
            nc.tensor.matmul(
                pot[:],
                OT[c][:, qb * 128 : (qb + 1) * 128],
                wo[c][:, nh * OW : (nh + 1) * OW],
                start=(c == 0),
                stop=(c == NHD - 1),
            )
        # ACT is exp-saturated while these run interleaved with attention;
        # only the final span's tail may borrow it
        if tail and (qb + nh) % 2 == 0:
            nc.scalar.copy(ob[:, nh * OW : (nh + 1) * OW], pot[:])
        else:
            nc.vector.tensor_copy(ob[:, nh * OW : (nh + 1) * OW], pot[:])
        if nh == NOUT - 1:
            nc.sync.dma_start(io["out"][qb * 128 : (qb + 1) * 128, :], ob[:])
            del ob_tiles[qb]

    def qkv_groups(sp):
        for ob in range(NQK):
            yield lambda ob=ob: pqk_group(sp, ob)
        for j in range(SBS):
            yield lambda j=j: pv_group(sp, j)

    def out_groups(sp, tail=False):
        for qb in range(sp * SBS, (sp + 1) * SBS):
            for nh in range(NOUT):
                yield lambda qb=qb, nh=nh: pot_group(sp, qb, nh, tail)

    def attention(sp, fillers):
        # ---- attention for this span (flash, S^T layout, head pairs) ----
        den = work.tile(
            [32 * (HPC - 1) + 1, SPAN], F32, name=f"den_{sp}", tag="den", bufs=1
        )
        nsb = (sp + 1) * SBS  # causal: sk blocks up to the span end
        pos = {}
        pts = {}
        oraw = {}

        def emit_scores(sb):
            for p in range(NPAIR):
                qt = qkT[p]
                kt = qkT[NQK // 2 + p]
                ps = psum.tile(
                    [128, 2 * SPAN], F32, name=f"ps_{p}_{sp}_{sb}", tag="ps", bufs=1
                )
                for i in range(2):
                    r = i * 64
                    nc.tensor.matmul(
                        ps[:, i * SPAN : (i + 1) * SPAN],
                        kt[r : r + 64, sb * 128 : (sb + 1) * 128],
                        qt[r : r + 64, sp * SPAN : (sp + 1) * SPAN],
                        start=True,
                        stop=True,
                    )
                pt = work.tile(
                    [128, 2 * SPAN], BF16, name=f"pt_{p}_{sp}_{sb}", tag="pt", bufs=4
                )
                pts[(p, sb)] = pt
                d = sb - sp * SBS
                if d < 0:
                    nc.scalar.activation(pt[:], ps[:], AF.Exp, scale=0.125)
                else:
                    # diagonal block: per head, cols < 128*d fully masked,
                    # then one triangular 128x128 sub-block
                    ptv = pt[:].rearrange("p (h w) -> p h w", w=SPAN)
                    psv = ps[:].rearrange("p (h w) -> p h w", w=SPAN)
                    if d > 0:
                        nc.vector.memset(ptv[:, :, 0 : 128 * d], 0.0)
                    nc.scalar.activation(
                        ptv[:, :, 128 * d : SPAN],
                        psv[:, :, 128 * d : SPAN],
                        AF.Exp,
                        scale=0.125,
                    )
                    for i in range(2):
                        nc.vector.tensor_mul(
                            pt[:, i * SPAN + 128 * d : i * SPAN + 128 * (d + 1)],
                            pt[:, i * SPAN + 128 * d : i * SPAN + 128 * (d + 1)],
                            tri[:],
                        )

        def emit_pvs(sb):
            for h in range(HPC):
                if sb == 0:
                    pos[h] = psum.tile(
                        [VW, SPAN], F32, name=f"po_{h}_{sp}", tag="po", bufs=4
                    )
                pt = pts[(h // 2, sb)]
                nc.tensor.matmul(
                    pos[h][:],
                    vp[sb][:, h * VW : (h + 1) * VW],
                    pt[:, (h % 2) * SPAN : (h % 2 + 1) * SPAN],
                    start=(sb == 0),
                    stop=(sb == nsb - 1),
                )
                if h % 2 == 1:
                    pts.pop((h // 2, sb))
                if sb == nsb - 1:
                    # denominator row straight from PSUM so the recip chain
                    # starts early; then out^T to SBUF to free the bank
                    nc.vector.tensor_copy(
                        den[32 * h : 32 * h + 1, :], pos[h][VW - 1 : VW, :]
                    )
                    orw = work.tile(
                        [VW, SPAN], F32, name=f"oraw_{h}_{sp}", tag="oraw", bufs=4
                    )
                    oraw[h] = orw
                    nc.vector.tensor_copy(orw[:], pos[h][:])

        fi = 0
        for i in range(nsb + 1):
            if i < nsb:
                emit_scores(i)
            if i >= 1:
                emit_pvs(i - 1)
            # sprinkle interleaved projection work evenly across the span
            want = (len(fillers) * (i + 1)) // (nsb + 1)
            while fi < want:
                fillers[fi]()
                fi += 1
        while fi < len(fillers):
            fillers[fi]()
            fi += 1

        denr = work.tile(
            [32 * (HPC - 1) + 1, SPAN], F32, name=f"denr_{sp}", tag="denr", bufs=1
        )
        # only rows 0/32/64/96 are meaningful; recip of the garbage rows in
        # between is never read. ~51 ULP accuracy is plenty for a softmax
        # denominator against the 2e-2 gate.
        nc.vector.reciprocal_approx_fast(denr[:], den[:])
        for h in range(HPC):
            ot_tile = OT[(h * DH) // 128]
            orow = (h * DH) % 128
            rtmp = work.tile([1, SPAN], F32, name=f"rtmp_{h}_{sp}", tag="rtmp", bufs=2)
            # partition_broadcast needs a partition-0 source
            nc.vector.tensor_copy(rtmp[:], denr[32 * h : 32 * h + 1, :])
            recb = work.tile([DH, SPAN], F32, name=f"recb_{h}_{sp}", tag="recb", bufs=2)
            nc.gpsimd.partition_broadcast(recb[:], rtmp[0:1, :])
            nc.vector.tensor_mul(
                ot_tile[orow : orow + DH, sp * SPAN : (sp + 1) * SPAN],
                oraw[h][0:DH, :],
                recb[:],
            )

    # ---- software pipeline over spans ----
    for g in qkv_groups(0):
        g()
    attention(0, list(qkv_groups(1)))
    attention(1, list(qkv_groups(2)) + list(out_groups(0)))
    attention(2, list(qkv_groups(3)) + list(out_groups(1)))
    attention(3, list(out_groups(2)))
    for g in out_groups(3, tail=True):
        g()


_NC_CACHE = {}


def _get_compiled():
    if "nc" not in _NC_CACHE:
        nc = bacc.Bacc(
            "TRN2", target_bir_lowering=False, debug=False, num_devices=N_CORES
        )
        io = _declare_io(nc)
        with tile.TileContext(nc) as tc, ExitStack() as ctx:
            _build(ctx, tc, io)
        nc.compile()
        _NC_CACHE["nc"] = nc
    return _NC_CACHE["nc"]


def _prep_core_inputs(x, W_qkv, b_qkv, W_out, b_out, core_id, tri):
    bf16 = ml_dtypes.bfloat16
    g = core_id // CPG
    lane = core_id % CPG
    h0 = lane * HPC
    r = slice(h0 * DH, (h0 + HPC) * DH)
    Wq = W_qkv[0 * DM : 1 * DM, :][r, :]
    Wk = W_qkv[1 * DM : 2 * DM, :][r, :]
    Wv = W_qkv[2 * DM : 3 * DM, :][r, :]
    bq = b_qkv[0 * DM + h0 * DH : 0 * DM + (h0 + HPC) * DH]
    bk = b_qkv[1 * DM + h0 * DH : 1 * DM + (h0 + HPC) * DH]
    bv_ = b_qkv[2 * DM + h0 * DH : 2 * DM + (h0 + HPC) * DH]
    return {
        "xT": np.ascontiguousarray(x[g].T.astype(bf16)),
        "wqkT": np.ascontiguousarray(
            np.concatenate([Wq.T, Wk.T], axis=1).astype(bf16)
        ),
        "wvT": np.ascontiguousarray(Wv.T.astype(bf16)),
        "woT": np.ascontiguousarray(W_out[:, r].T.astype(bf16)),
        "bqk": np.concatenate([bq, bk]).reshape(2 * DQ, 1).astype(np.float32),
        "bv": np.ascontiguousarray(
            np.broadcast_to(bv_.reshape(1, DQ), (128, DQ)).astype(np.float32)
        ),
        "tri": tri,
    }


def kernel(x, W_qkv, b_qkv, W_out, b_out, _trace=False):
    x = np.asarray(x)
    W_qkv = np.asarray(W_qkv)
    b_qkv = np.asarray(b_qkv)
    W_out = np.asarray(W_out)
    b_out = np.asarray(b_out)

    # tri[r, c] = (c >= r): keep (k, q) where q >= k in the diagonal block
    tri = np.triu(np.ones((128, 128), dtype=np.float32)).astype(ml_dtypes.bfloat16)

    nc = _get_compiled()
    in_maps = [
        _prep_core_inputs(x, W_qkv, b_qkv, W_out, b_out, c, tri)
        for c in range(N_CORES)
    ]
    res = run_bass_kernel_spmd(nc, in_maps, list(range(N_CORES)), trace=_trace)

    out = np.empty((B, S, DM), dtype=np.float32)
    for g in range(B):
        acc = res.results[g * CPG]["out"].astype(np.float32)
        for lane in range(1, CPG):
            acc = acc + res.results[g * CPG + lane]["out"].astype(np.float32)
        out[g] = acc + b_out[None, :].astype(np.float32)

    if _trace:
        kernel.last_exec_time_ns = res.exec_time_ns
        kernel.last_results = res
    return out


# revision 9
# speedup vs baseline: 1.7142x; 1.1302x over previous
"""Multi-head causal self-attention (torch nn.MultiheadAttention semantics)
on 8 Trainium2 NeuronCores.

Problem: x [2, 2048, 1024], 16 heads, head dim 64, fp32, causal, p_drop=0.

Sharding: 2 batch groups x 4-way head tensor-parallel.
  core c: batch b = c // 4, heads [lane*4, lane*4+4) with lane = c % 4.
Each core computes q/k/v projections for its 4 heads, flash-style causal
attention (S^T score layout, no-max softmax — scores are O(1) here), and its
partial out-projection. The host sums the 4 partials per batch and adds b_out
(the all-reduce of the tensor-parallel decomposition, free on host since the
harness contract is full-input -> full-output).

All matmuls run in bf16 (1 cycle/row on the PE, FWL weight loads) with fp32
PSUM accumulation; rel err vs the fp32 reference lands ~4e-3 against the
2e-2 gate.

Engine balance: the attention inner loop is ACT-bound (exp at 1 elem/cycle/
lane + ~293ns/instruction overhead), while the projections are PE-bound. So
the q/k/v projection of span sp+1 and the out-projection of span sp-1 are
emitted as small matmul groups INTERLEAVED into span sp's attention block
loop — the PE fills its exp-wait gaps with projection work and never idles
>3.4us (which would re-throttle the HAM clock gate to 1.2 GHz).

Attention per span, per sk-block: score matmuls for a head PAIR are emitted
back-to-back with lhsT base partitions 0/64 -> auto tile_position (0,0) /
(64,0) -> the two K=64 matmuls run concurrently in disjoint PE row-groups
(measured dstart ~4ns). Each pair writes one [128, 1024] 2-bank PSUM tile;
ONE merged ACTIVATE exps both heads' scores. Diagonal blocks memset the
fully-masked prefix and exp only the live region via a strided per-head
view, then multiply the diagonal 128x128 sub-block by a host-provided 0/1
triangular mask.

v' per sk-block: [128, 4*(64+1)] — per-head v with an appended ones column,
so the PV matmul's row 64 accumulates the softmax denominator for free.
out^T psum [65, 512] accumulates v'.T @ P over sk blocks; row 64 = denom;
normalized via reciprocal_approx_fast + gpsimd partition-broadcast + DVE
mul. out [S, DM] partial = OT.T @ woT per 128-row block, written back as
bf16; the host sums the 4 bf16 partials per batch in fp32 and adds b_out.

PSUM (8 banks): ps pair-score tiles [128,1024] x1 buf = 2 banks; po PV
accumulators [65,512] x4 = 4 banks; pp projection tiles [128,512] x2 = 2.
"""

from contextlib import ExitStack

import numpy as np
import ml_dtypes

import concourse.bass as bass
import concourse.tile as tile
from concourse import bacc, mybir
from concourse.bass_utils import run_bass_kernel_spmd

F32 = mybir.dt.float32
BF16 = mybir.dt.bfloat16
AF = mybir.ActivationFunctionType

B = 2
S = 2048
DM = 1024
N_HEADS = 16
DH = 64
N_CORES = 8
CPG = 4  # cores per group (tensor-parallel width over heads)
HPC = N_HEADS // CPG  # heads per core
DQ = HPC * DH
SPAN = 512
SB = 128
NDM = DM // 128
NSPAN = S // SPAN
NSB = S // SB
SBS = SPAN // SB
NQK = 2 * DQ // 128
NHD = DQ // 128
VW = DH + 1
OW = min(512, DM)
NOUT = DM // OW
NPAIR = HPC // 2


def _declare_io(nc):
    t = {}
    t["xT"] = nc.dram_tensor("xT", [DM, S], BF16, kind="ExternalInput").ap()
    t["wqkT"] = nc.dram_tensor("wqkT", [DM, 2 * DQ], BF16, kind="ExternalInput").ap()
    t["wvT"] = nc.dram_tensor("wvT", [DM, DQ], BF16, kind="ExternalInput").ap()
    t["woT"] = nc.dram_tensor("woT", [DQ, DM], BF16, kind="ExternalInput").ap()
    t["bqk"] = nc.dram_tensor("bqk", [2 * DQ, 1], F32, kind="ExternalInput").ap()
    t["bv"] = nc.dram_tensor("bv", [128, DQ], F32, kind="ExternalInput").ap()
    t["tri"] = nc.dram_tensor("tri", [128, 128], BF16, kind="ExternalInput").ap()
    t["out"] = nc.dram_tensor("out", [S, DM], BF16, kind="ExternalOutput").ap()
    return t


def _build(ctx: ExitStack, tc: tile.TileContext, io: dict):
    nc = tc.nc

    const = ctx.enter_context(tc.tile_pool(name="const", bufs=1))
    work = ctx.enter_context(tc.tile_pool(name="work", bufs=1))
    psum = ctx.enter_context(tc.tile_pool(name="psum", bufs=1, space="PSUM"))

    # ---- DMA order: minimum prefix for qk-proj of span 0 (wqk + xT sp0)
    # first, then the rest roughly in first-use order ----
    wqk = [const.tile([128, 2 * DQ], BF16, name=f"wqk{c}") for c in range(NDM)]
    for c in range(NDM):
        nc.sync.dma_start(wqk[c][:], io["wqkT"][c * 128 : (c + 1) * 128, :])

    xT = [[None] * NSPAN for _ in range(NDM)]

    def dma_x_span(sp):
        for c in range(NDM):
            xT[c][sp] = const.tile([128, SPAN], BF16, name=f"xT{c}_{sp}")
            nc.sync.dma_start(
                xT[c][sp][:],
                io["xT"][c * 128 : (c + 1) * 128, sp * SPAN : (sp + 1) * SPAN],
            )

    dma_x_span(0)
    wv = [const.tile([128, DQ], BF16, name=f"wv{c}") for c in range(NDM)]
    for c in range(NDM):
        nc.sync.dma_start(wv[c][:], io["wvT"][c * 128 : (c + 1) * 128, :])

    bqk = [const.tile([128, 1], F32, name=f"bqk{c}") for c in range(NQK)]
    for c in range(NQK):
        nc.sync.dma_start(bqk[c][:], io["bqk"][c * 128 : (c + 1) * 128, :])
    bv = const.tile([128, DQ], F32, name="bv")
    nc.sync.dma_start(bv[:], io["bv"][:])
    tri = const.tile([128, 128], BF16, name="tri")
    nc.sync.dma_start(tri[:], io["tri"][:])

    dma_x_span(1)
    wo = [const.tile([128, DM], BF16, name=f"wo{c}") for c in range(NHD)]
    for c in range(NHD):
        nc.sync.dma_start(wo[c][:], io["woT"][c * 128 : (c + 1) * 128, :])
    dma_x_span(2)
    dma_x_span(3)

    qkT = [const.tile([128, S], BF16, name=f"qkT{b}") for b in range(NQK)]
    vp = [const.tile([128, HPC * VW], BF16, name=f"vp{sb}") for sb in range(NSB)]
    OT = [const.tile([128, S], BF16, name=f"OT{c}") for c in range(NHD)]

    # ---- projection / out-projection emitters, one small PE group each,
    # suitable for interleaving into the attention block loop ----
    def pqk_group(sp, ob):
        pqk = psum.tile([128, SPAN], F32, name=f"pqk_{ob}_{sp}", tag="pp", bufs=2)
        for c in range(NDM):
            nc.tensor.matmul(
                pqk[:],
                wqk[c][:, ob * 128 : (ob + 1) * 128],
                xT[c][sp][:],
                start=(c == 0),
                stop=(c == NDM - 1),
            )
        nc.vector.tensor_scalar_add(
            qkT[ob][:, sp * SPAN : (sp + 1) * SPAN], pqk[:], bqk[ob][:]
        )

    def pv_group(sp, j):
        sb = sp * SBS + j
        pv = psum.tile([128, DQ], F32, name=f"pv_{sb}", tag="pp", bufs=2)
        for c in range(NDM):
            nc.tensor.matmul(
                pv[:],
                xT[c][sp][:, j * 128 : (j + 1) * 128],
                wv[c][:],
                start=(c == 0),
                stop=(c == NDM - 1),
            )
        vdst = vp[sb][:, 0 : HPC * VW].rearrange("p (h w) -> p h w", w=VW)[:, :, 0:DH]
        nc.vector.tensor_add(
            vdst,
            pv[:].rearrange("p (h d) -> p h d", d=DH),
            bv[:].rearrange("p (h d) -> p h d", d=DH),
        )
        ones_cols = vp[sb][:, DH : HPC * VW : VW]
        nc.vector.memset(ones_cols, 1.0)

    ob_tiles = {}

    def pot_group(sp, qb, nh, tail=False):
        if nh == 0:
            ob_tiles[qb] = work.tile([128, DM], BF16, name=f"ob_{qb}", tag="ob", bufs=2)
        ob = ob_tiles[qb]
        pot = psum.tile([128, OW], F32, name=f"pot_{qb}_{nh}", tag="pp", bufs=2)
        for c in range(NHD):
            nc.tensor.matmul(
                pot[:],
                OT[c][:, qb * 128 : (qb + 1) * 128],
                wo[c][:, nh * OW : (nh + 1) * OW],
                start=(c == 0),
                stop=(c == NHD - 1),
            )
        # ACT is exp-saturated while these run interleaved with attention;
        # only the final span's tail may borrow it
        if tail and (qb + nh) % 2 == 0:
            nc.scalar.copy(ob[:, nh * OW : (nh + 1) * OW], pot[:])
        else:
            nc.vector.tensor_copy(ob[:, nh * OW : (nh + 1) * OW], pot[:])
        if nh == NOUT - 1:
            nc.sync.dma_start(io["out"][qb * 128 : (qb + 1) * 128, :], ob[:])
            del ob_tiles[qb]

    def qkv_groups(sp):
        for ob in range(NQK):
            yield lambda ob=ob: pqk_group(sp, ob)
        for j in range(SBS):
            yield lambda j=j: pv_group(sp, j)

    def out_groups(sp, tail=False):
        for qb in range(sp * SBS, (sp + 1) * SBS):
            for nh in range(NOUT):
                yield lambda qb=qb, nh=nh: pot_group(sp, qb, nh, tail)

    def attention(sp, fillers):
        # ---- attention for this span (flash, S^T layout) ----
        # Head pairs are TIME-multiplexed (pair 0 over all sk blocks, then
        # pair 1) so only 2 PV accumulator banks are live at once, freeing
        # PSUM for double-buffered score tiles + interleaved projections.
        den = work.tile(
            [32 * (HPC - 1) + 1, SPAN], F32, name=f"den_{sp}", tag="den", bufs=1
        )
        nsb = (sp + 1) * SBS  # causal: sk blocks up to the span end
        oraw = {}
        fi = 0
        it = 0
        total_iters = NPAIR * (nsb + 1)

        for p in range(NPAIR):
            qt = qkT[p]
            kt = qkT[NQK // 2 + p]
            pos = {}
            pts = {}

            def emit_scores(sb):
                ps = psum.tile(
                    [128, 2 * SPAN], F32, name=f"ps_{p}_{sp}_{sb}", tag="ps", bufs=2
                )
                for i in range(2):
                    r = i * 64
                    nc.tensor.matmul(
                        ps[:, i * SPAN : (i + 1) * SPAN],
                        kt[r : r + 64, sb * 128 : (sb + 1) * 128],
                        qt[r : r + 64, sp * SPAN : (sp + 1) * SPAN],
                        start=True,
                        stop=True,
                    )
                pt = work.tile(
                    [128, 2 * SPAN], BF16, name=f"pt_{p}_{sp}_{sb}", tag="pt", bufs=4
                )
                pts[sb] = pt
                d = sb - sp * SBS
                if d < 0:
                    nc.scalar.activation(pt[:], ps[:], AF.Exp, scale=0.125)
                else:
                    # diagonal block: per head, cols < 128*d fully masked,
                    # then one triangular 128x128 sub-block
                    ptv = pt[:].rearrange("p (h w) -> p h w", w=SPAN)
                    psv = ps[:].rearrange("p (h w) -> p h w", w=SPAN)
                    if d > 0:
                        nc.vector.memset(ptv[:, :, 0 : 128 * d], 0.0)
                    nc.scalar.activation(
                        ptv[:, :, 128 * d : SPAN],
                        psv[:, :, 128 * d : SPAN],
                        AF.Exp,
                        scale=0.125,
                    )
                    for i in range(2):
                        nc.vector.tensor_mul(
                            pt[:, i * SPAN + 128 * d : i * SPAN + 128 * (d + 1)],
                            pt[:, i * SPAN + 128 * d : i * SPAN + 128 * (d + 1)],
                            tri[:],
                        )

            def emit_pvs(sb):
                for h in (2 * p, 2 * p + 1):
                    if sb == 0:
                        pos[h] = psum.tile(
                            [VW, SPAN], F32, name=f"po_{h}_{sp}", tag="po", bufs=2
                        )
                    pt = pts[sb]
                    nc.tensor.matmul(
                        pos[h][:],
                        vp[sb][:, h * VW : (h + 1) * VW],
                        pt[:, (h % 2) * SPAN : (h % 2 + 1) * SPAN],
                        start=(sb == 0),
                        stop=(sb == nsb - 1),
                    )
                    if h % 2 == 1:
                        pts.pop(sb)
                    if sb == nsb - 1:
                        # denominator row straight from PSUM so the recip
                        # chain starts early; then out^T to SBUF to free
                        # the bank
                        nc.vector.tensor_copy(
                            den[32 * h : 32 * h + 1, :], pos[h][VW - 1 : VW, :]
                        )
                        orw = work.tile(
                            [VW, SPAN], F32, name=f"oraw_{h}_{sp}", tag="oraw",
                            bufs=4,
                        )
                        oraw[h] = orw
                        nc.vector.tensor_copy(orw[:], pos[h][:])

            for i in range(nsb + 1):
                if i < nsb:
                    emit_scores(i)
                if i >= 1:
                    emit_pvs(i - 1)
                it += 1
                # sprinkle interleaved projection work evenly across the span
                want = (len(fillers) * it) // total_iters
                while fi < want:
                    fillers[fi]()
                    fi += 1
        while fi < len(fillers):
            fillers[fi]()
            fi += 1

        denr = work.tile(
            [32 * (HPC - 1) + 1, SPAN], F32, name=f"denr_{sp}", tag="denr", bufs=1
        )
        # only rows 0/32/64/96 are meaningful; recip of the garbage rows in
        # between is never read. ~51 ULP accuracy is plenty for a softmax
        # denominator against the 2e-2 gate.
        nc.vector.reciprocal_approx_fast(denr[:], den[:])
        for h in range(HPC):
            ot_tile = OT[(h * DH) // 128]
            orow = (h * DH) % 128
            rtmp = work.tile([1, SPAN], F32, name=f"rtmp_{h}_{sp}", tag="rtmp", bufs=2)
            # partition_broadcast needs a partition-0 source
            nc.vector.tensor_copy(rtmp[:], denr[32 * h : 32 * h + 1, :])
            recb = work.tile([DH, SPAN], F32, name=f"recb_{h}_{sp}", tag="recb", bufs=2)
            nc.gpsimd.partition_broadcast(recb[:], rtmp[0:1, :])
            nc.vector.tensor_mul(
                ot_tile[orow : orow + DH, sp * SPAN : (sp + 1) * SPAN],
                oraw[h][0:DH, :],
                recb[:],
            )

    # ---- software pipeline over spans ----
    for g in qkv_groups(0):
        g()
    attention(0, list(qkv_groups(1)))
    attention(1, list(qkv_groups(2)) + list(out_groups(0)))
    attention(2, list(qkv_groups(3)) + list(out_groups(1)))
    attention(3, list(out_groups(2)))
    for g in out_groups(3, tail=True):
        g()


_NC_CACHE = {}


def _get_compiled():
    if "nc" not in _NC_CACHE:
        nc = bacc.Bacc(
            "TRN2", target_bir_lowering=False, debug=False, num_devices=N_CORES
        )
        io = _declare_io(nc)
        with tile.TileContext(nc) as tc, ExitStack() as ctx:
            _build(ctx, tc, io)
        nc.compile()
        _NC_CACHE["nc"] = nc
    return _NC_CACHE["nc"]


def _prep_core_inputs(x, W_qkv, b_qkv, W_out, b_out, core_id, tri):
    bf16 = ml_dtypes.bfloat16
    g = core_id // CPG
    lane = core_id % CPG
    h0 = lane * HPC
    r = slice(h0 * DH, (h0 + HPC) * DH)
    Wq = W_qkv[0 * DM : 1 * DM, :][r, :]
    Wk = W_qkv[1 * DM : 2 * DM, :][r, :]
    Wv = W_qkv[2 * DM : 3 * DM, :][r, :]
    bq = b_qkv[0 * DM + h0 * DH : 0 * DM + (h0 + HPC) * DH]
    bk = b_qkv[1 * DM + h0 * DH : 1 * DM + (h0 + HPC) * DH]
    bv_ = b_qkv[2 * DM + h0 * DH : 2 * DM + (h0 + HPC) * DH]
    return {
        "xT": np.ascontiguousarray(x[g].T.astype(bf16)),
        "wqkT": np.ascontiguousarray(
            np.concatenate([Wq.T, Wk.T], axis=1).astype(bf16)
        ),
        "wvT": np.ascontiguousarray(Wv.T.astype(bf16)),
        "woT": np.ascontiguousarray(W_out[:, r].T.astype(bf16)),
        "bqk": np.concatenate([bq, bk]).reshape(2 * DQ, 1).astype(np.float32),
        "bv": np.ascontiguousarray(
            np.broadcast_to(bv_.reshape(1, DQ), (128, DQ)).astype(np.float32)
        ),
        "tri": tri,
    }


def kernel(x, W_qkv, b_qkv, W_out, b_out, _trace=False):
    x = np.asarray(x)
    W_qkv = np.asarray(W_qkv)
    b_qkv = np.asarray(b_qkv)
    W_out = np.asarray(W_out)
    b_out = np.asarray(b_out)

    # tri[r, c] = (c >= r): keep (k, q) where q >= k in the diagonal block
    tri = np.triu(np.ones((128, 128), dtype=np.float32)).astype(ml_dtypes.bfloat16)

    nc = _get_compiled()
    in_maps = [
        _prep_core_inputs(x, W_qkv, b_qkv, W_out, b_out, c, tri)
        for c in range(N_CORES)
    ]
    res = run_bass_kernel_spmd(nc, in_maps, list(range(N_CORES)), trace=_trace)

    out = np.empty((B, S, DM), dtype=np.float32)
    for g in range(B):
        acc = res.results[g * CPG]["out"].astype(np.float32)
        for lane in range(1, CPG):
            acc = acc + res.results[g * CPG + lane]["out"].astype(np.float32)
        out[g] = acc + b_out[None, :].astype(np.float32)

    if _trace:
        kernel.last_exec_time_ns = res.exec_time_ns
        kernel.last_results = res
    return out


# revision 14
# speedup vs baseline: 1.7800x; 1.0384x over previous
"""Multi-head causal self-attention (torch nn.MultiheadAttention semantics)
on 8 Trainium2 NeuronCores.

Problem: x [2, 2048, 1024], 16 heads, head dim 64, fp32, causal, p_drop=0.

Sharding: 2 batch groups x 4-way head tensor-parallel.
  core c: batch b = c // 4, heads [lane*4, lane*4+4) with lane = c % 4.
Each core computes q/k/v projections for its 4 heads, flash-style causal
attention (S^T score layout, no-max softmax — scores are O(1) here), and its
partial out-projection. The host sums the 4 partials per batch and adds b_out
(the all-reduce of the tensor-parallel decomposition, free on host since the
harness contract is full-input -> full-output).

All matmuls run in bf16 (1 cycle/row on the PE, FWL weight loads) with fp32
PSUM accumulation; rel err vs the fp32 reference lands ~4e-3 against the
2e-2 gate.

Engine balance: the attention inner loop is ACT-bound (exp at 1 elem/cycle/
lane + ~293ns/instruction overhead), while the projections are PE-bound. So
the q/k/v projection of span sp+1 and the out-projection of span sp-1 are
emitted as small matmul groups INTERLEAVED into span sp's attention block
loop — the PE fills its exp-wait gaps with projection work and never idles
>3.4us (which would re-throttle the HAM clock gate to 1.2 GHz).

Attention per span, per sk-block: score matmuls for a head PAIR are emitted
back-to-back with lhsT base partitions 0/64 -> auto tile_position (0,0) /
(64,0) -> the two K=64 matmuls run concurrently in disjoint PE row-groups
(measured dstart ~4ns). Each pair writes one [128, 1024] 2-bank PSUM tile;
ONE merged ACTIVATE exps both heads' scores. Diagonal blocks memset the
fully-masked prefix and exp only the live region via a strided per-head
view, then multiply the diagonal 128x128 sub-block by a host-provided 0/1
triangular mask.

v' per sk-block: [128, 4*(64+1)] — per-head v with an appended ones column,
so the PV matmul's row 64 accumulates the softmax denominator for free.
out^T psum [65, 512] accumulates v'.T @ P over sk blocks; row 64 = denom;
normalized via reciprocal_approx_fast + gpsimd partition-broadcast + DVE
mul. out [S, DM] partial = OT.T @ woT per 128-row block, written back as
bf16; the host sums the 4 bf16 partials per batch in fp32 and adds b_out.

PSUM (8 banks): ps pair-score tiles [128,1024] x1 buf = 2 banks; po PV
accumulators [65,512] x4 = 4 banks; pp projection tiles [128,512] x2 = 2.
"""

from contextlib import ExitStack

import numpy as np
import ml_dtypes

import concourse.bass as bass
import concourse.tile as tile
from concourse import bacc, mybir
from concourse.bass_utils import run_bass_kernel_spmd

F32 = mybir.dt.float32
BF16 = mybir.dt.bfloat16
AF = mybir.ActivationFunctionType

B = 2
S = 2048
DM = 1024
N_HEADS = 16
DH = 64
N_CORES = 8
CPG = 4  # cores per group (tensor-parallel width over heads)
HPC = N_HEADS // CPG  # heads per core
DQ = HPC * DH
SPAN = 512
SB = 128
NDM = DM // 128
NSPAN = S // SPAN
NSB = S // SB
SBS = SPAN // SB
NQK = 2 * DQ // 128
NHD = DQ // 128
VW = DH + 1
OW = min(512, DM)
NOUT = DM // OW
NPAIR = HPC // 2


def _declare_io(nc):
    t = {}
    t["xT"] = nc.dram_tensor("xT", [DM, S], BF16, kind="ExternalInput").ap()
    t["wqkT"] = nc.dram_tensor("wqkT", [DM, 2 * DQ], BF16, kind="ExternalInput").ap()
    t["wvT"] = nc.dram_tensor("wvT", [DM, DQ], BF16, kind="ExternalInput").ap()
    t["woT"] = nc.dram_tensor("woT", [DQ, DM], BF16, kind="ExternalInput").ap()
    t["bqk"] = nc.dram_tensor("bqk", [2 * DQ, 1], F32, kind="ExternalInput").ap()
    t["bv"] = nc.dram_tensor("bv", [128, DQ], F32, kind="ExternalInput").ap()
    t["tri"] = nc.dram_tensor("tri", [128, 128], BF16, kind="ExternalInput").ap()
    t["out"] = nc.dram_tensor("out", [S, DM], BF16, kind="ExternalOutput").ap()
    return t


def _build(ctx: ExitStack, tc: tile.TileContext, io: dict):
    nc = tc.nc

    const = ctx.enter_context(tc.tile_pool(name="const", bufs=1))
    work = ctx.enter_context(tc.tile_pool(name="work", bufs=1))
    psum = ctx.enter_context(tc.tile_pool(name="psum", bufs=1, space="PSUM"))

    # ---- inputs: ONE DMA instruction per tensor/span (the Sync engine
    # dispatches dma_starts serially at ~600ns each — many small DMAs
    # serialize the whole input load). dm-chunks are packed side by side
    # in the free dim via a rearranged 3D DRAM access pattern. Order:
    # minimum prefix for qk-proj of span 0 first. ----
    wqkt = const.tile([128, NDM * 2 * DQ], BF16, name="wqkt")
    nc.sync.dma_start(
        wqkt[:].rearrange("p (c w) -> p c w", w=2 * DQ),
        io["wqkT"].rearrange("(c p) w -> p c w", p=128),
    )

    xsp = [const.tile([128, NDM * SPAN], BF16, name=f"xsp{sp}") for sp in range(NSPAN)]

    def dma_x_span(sp):
        nc.sync.dma_start(
            xsp[sp][:].rearrange("p (c s) -> p c s", s=SPAN),
            io["xT"].rearrange("(c p) s -> p c s", p=128)[
                :, :, sp * SPAN : (sp + 1) * SPAN
            ],
        )

    def xt(c, sp):
        return xsp[sp][:, c * SPAN : (c + 1) * SPAN]

    dma_x_span(0)
    wvt = const.tile([128, NDM * DQ], BF16, name="wvt")
    nc.sync.dma_start(
        wvt[:].rearrange("p (c w) -> p c w", w=DQ),
        io["wvT"].rearrange("(c p) w -> p c w", p=128),
    )

    bqkt = const.tile([128, NQK], F32, name="bqkt")
    nc.sync.dma_start(
        bqkt[:].rearrange("p (c o) -> p c o", o=1),
        io["bqk"].rearrange("(c p) o -> p c o", p=128),
    )
    bv = const.tile([128, DQ], F32, name="bv")
    nc.sync.dma_start(bv[:], io["bv"][:])
    tri = const.tile([128, 128], BF16, name="tri")
    nc.sync.dma_start(tri[:], io["tri"][:])

    dma_x_span(1)
    wot = const.tile([128, NHD * DM], BF16, name="wot")
    nc.sync.dma_start(
        wot[:].rearrange("p (c w) -> p c w", w=DM),
        io["woT"].rearrange("(c p) w -> p c w", p=128),
    )
    dma_x_span(2)
    dma_x_span(3)

    qkT = [const.tile([128, S], BF16, name=f"qkT{b}") for b in range(NQK)]
    vp = [const.tile([128, HPC * VW], BF16, name=f"vp{sb}") for sb in range(NSB)]
    OT = [const.tile([128, S], BF16, name=f"OT{c}") for c in range(NHD)]

    # ---- projection / out-projection emitters, one small PE group each,
    # suitable for interleaving into the attention block loop ----
    def pqk_group(sp, ob):
        pqk = psum.tile([128, SPAN], F32, name=f"pqk_{ob}_{sp}", tag="pp", bufs=2)
        for c in range(NDM):
            nc.tensor.matmul(
                pqk[:],
                wqkt[:, c * 2 * DQ + ob * 128 : c * 2 * DQ + (ob + 1) * 128],
                xt(c, sp),
                start=(c == 0),
                stop=(c == NDM - 1),
            )
        nc.vector.tensor_scalar_add(
            qkT[ob][:, sp * SPAN : (sp + 1) * SPAN], pqk[:], bqkt[:, ob : ob + 1]
        )

    def pv_group(sp, j):
        sb = sp * SBS + j
        pv = psum.tile([128, DQ], F32, name=f"pv_{sb}", tag="pp", bufs=2)
        for c in range(NDM):
            nc.tensor.matmul(
                pv[:],
                xt(c, sp)[:, j * 128 : (j + 1) * 128],
                wvt[:, c * DQ : (c + 1) * DQ],
                start=(c == 0),
                stop=(c == NDM - 1),
            )
        vdst = vp[sb][:, 0 : HPC * VW].rearrange("p (h w) -> p h w", w=VW)[:, :, 0:DH]
        nc.vector.tensor_add(
            vdst,
            pv[:].rearrange("p (h d) -> p h d", d=DH),
            bv[:].rearrange("p (h d) -> p h d", d=DH),
        )
        ones_cols = vp[sb][:, DH : HPC * VW : VW]
        nc.vector.memset(ones_cols, 1.0)

    ob_tiles = {}

    def pot_group(sp, qb, nh, tail=False):
        if nh == 0:
            ob_tiles[qb] = work.tile([128, DM], BF16, name=f"ob_{qb}", tag="ob", bufs=2)
        ob = ob_tiles[qb]
        pot = psum.tile([128, OW], F32, name=f"pot_{qb}_{nh}", tag="pp", bufs=2)
        for c in range(NHD):
            nc.tensor.matmul(
                pot[:],
                OT[c][:, qb * 128 : (qb + 1) * 128],
                wot[:, c * DM + nh * OW : c * DM + (nh + 1) * OW],
                start=(c == 0),
                stop=(c == NHD - 1),
            )
        # ACT is exp-saturated while these run interleaved with attention;
        # only the final span's tail may borrow it
        if tail and (qb + nh) % 2 == 0:
            nc.scalar.copy(ob[:, nh * OW : (nh + 1) * OW], pot[:])
        else:
            nc.vector.tensor_copy(ob[:, nh * OW : (nh + 1) * OW], pot[:])
        if nh == NOUT - 1:
            nc.sync.dma_start(io["out"][qb * 128 : (qb + 1) * 128, :], ob[:])
            del ob_tiles[qb]

    def qkv_groups(sp):
        for ob in range(NQK):
            yield lambda ob=ob: pqk_group(sp, ob)
        for j in range(SBS):
            yield lambda j=j: pv_group(sp, j)

    def out_groups(sp, tail=False):
        for qb in range(sp * SBS, (sp + 1) * SBS):
            for nh in range(NOUT):
                yield lambda qb=qb, nh=nh: pot_group(sp, qb, nh, tail)

    def attention(sp, fillers):
        # ---- attention for this span (flash, S^T layout) ----
        # Head pairs are TIME-multiplexed (pair 0 over all sk blocks, then
        # pair 1) so only 2 PV accumulator banks are live at once, freeing
        # PSUM for double-buffered score tiles + interleaved projections.
        den = work.tile(
            [32 * (HPC - 1) + 1, SPAN], F32, name=f"den_{sp}", tag="den", bufs=1
        )
        nsb = (sp + 1) * SBS  # causal: sk blocks up to the span end
        oraw = {}
        fi = 0
        it = 0
        total_iters = NPAIR * (nsb + 1)

        for p in range(NPAIR):
            qt = qkT[p]
            kt = qkT[NQK // 2 + p]
            pos = {}
            pts = {}

            def emit_scores(sb):
                ps = psum.tile(
                    [128, 2 * SPAN], F32, name=f"ps_{p}_{sp}_{sb}", tag="ps", bufs=2
                )
                for i in range(2):
                    r = i * 64
                    nc.tensor.matmul(
                        ps[:, i * SPAN : (i + 1) * SPAN],
                        kt[r : r + 64, sb * 128 : (sb + 1) * 128],
                        qt[r : r + 64, sp * SPAN : (sp + 1) * SPAN],
                        start=True,
                        stop=True,
                    )
                pt = work.tile(
                    [128, 2 * SPAN], BF16, name=f"pt_{p}_{sp}_{sb}", tag="pt", bufs=4
                )
                pts[sb] = pt
                d = sb - sp * SBS
                if d < 0:
                    nc.scalar.activation(pt[:], ps[:], AF.Exp, scale=0.125)
                else:
                    # diagonal block: per head, cols < 128*d fully masked,
                    # then one triangular 128x128 sub-block
                    ptv = pt[:].rearrange("p (h w) -> p h w", w=SPAN)
                    psv = ps[:].rearrange("p (h w) -> p h w", w=SPAN)
                    if d > 0:
                        nc.vector.memset(ptv[:, :, 0 : 128 * d], 0.0)
                    nc.scalar.activation(
                        ptv[:, :, 128 * d : SPAN],
                        psv[:, :, 128 * d : SPAN],
                        AF.Exp,
                        scale=0.125,
                    )
                    for i in range(2):
                        nc.vector.tensor_mul(
                            pt[:, i * SPAN + 128 * d : i * SPAN + 128 * (d + 1)],
                            pt[:, i * SPAN + 128 * d : i * SPAN + 128 * (d + 1)],
                            tri[:],
                        )

            def emit_pvs(sb):
                for h in (2 * p, 2 * p + 1):
                    if sb == 0:
                        pos[h] = psum.tile(
                            [VW, SPAN], F32, name=f"po_{h}_{sp}", tag="po", bufs=2
                        )
                    pt = pts[sb]
                    nc.tensor.matmul(
                        pos[h][:],
                        vp[sb][:, h * VW : (h + 1) * VW],
                        pt[:, (h % 2) * SPAN : (h % 2 + 1) * SPAN],
                        start=(sb == 0),
                        stop=(sb == nsb - 1),
                    )
                    if h % 2 == 1:
                        pts.pop(sb)
                    if sb == nsb - 1:
                        # denominator row straight from PSUM so the recip
                        # chain starts early; then out^T to SBUF to free
                        # the bank
                        nc.vector.tensor_copy(
                            den[32 * h : 32 * h + 1, :], pos[h][VW - 1 : VW, :]
                        )
                        orw = work.tile(
                            [VW, SPAN], F32, name=f"oraw_{h}_{sp}", tag="oraw",
                            bufs=4,
                        )
                        oraw[h] = orw
                        nc.vector.tensor_copy(orw[:], pos[h][:])

            for i in range(nsb + 1):
                if i < nsb:
                    emit_scores(i)
                if i >= 1:
                    emit_pvs(i - 1)
                it += 1
                if i == nsb:
                    # pair boundary: defer fillers so the next pair's score
                    # matmuls (which feed the exp-starved ACT) issue first
                    continue
                # sprinkle interleaved projection work evenly across the span
                want = (len(fillers) * it) // total_iters
                while fi < want:
                    fillers[fi]()
                    fi += 1
        while fi < len(fillers):
            fillers[fi]()
            fi += 1

        denr = work.tile(
            [32 * (HPC - 1) + 1, SPAN], F32, name=f"denr_{sp}", tag="denr", bufs=1
        )
        # only rows 0/32/64/96 are meaningful; recip of the garbage rows in
        # between is never read. ~51 ULP accuracy is plenty for a softmax
        # denominator against the 2e-2 gate.
        nc.vector.reciprocal_approx_fast(denr[:], den[:])
        for h in range(HPC):
            ot_tile = OT[(h * DH) // 128]
            orow = (h * DH) % 128
            rtmp = work.tile([1, SPAN], F32, name=f"rtmp_{h}_{sp}", tag="rtmp", bufs=2)
            # partition_broadcast needs a partition-0 source
            nc.vector.tensor_copy(rtmp[:], denr[32 * h : 32 * h + 1, :])
            recb = work.tile([DH, SPAN], F32, name=f"recb_{h}_{sp}", tag="recb", bufs=2)
            nc.gpsimd.partition_broadcast(recb[:], rtmp[0:1, :])
            nc.vector.tensor_mul(
                ot_tile[orow : orow + DH, sp * SPAN : (sp + 1) * SPAN],
                oraw[h][0:DH, :],
                recb[:],
            )

    # ---- software pipeline over spans ----
    for g in qkv_groups(0):
        g()
    attention(0, list(qkv_groups(1)))
    attention(1, list(qkv_groups(2)) + list(out_groups(0)))
    attention(2, list(qkv_groups(3)) + list(out_groups(1)))
    attention(3, list(out_groups(2)))
    for g in out_groups(3, tail=True):
        g()


_NC_CACHE = {}


def _get_compiled():
    if "nc" not in _NC_CACHE:
        nc = bacc.Bacc(
            "TRN2", target_bir_lowering=False, debug=False, num_devices=N_CORES
        )
        io = _declare_io(nc)
        with tile.TileContext(nc) as tc, ExitStack() as ctx:
            _build(ctx, tc, io)
        nc.compile()
        _NC_CACHE["nc"] = nc
    return _NC_CACHE["nc"]


def _prep_core_inputs(x, W_qkv, b_qkv, W_out, b_out, core_id, tri):
    bf16 = ml_dtypes.bfloat16
    g = core_id // CPG
    lane = core_id % CPG
    h0 = lane * HPC
    r = slice(h0 * DH, (h0 + HPC) * DH)
    Wq = W_qkv[0 * DM : 1 * DM, :][r, :]
    Wk = W_qkv[1 * DM : 2 * DM, :][r, :]
    Wv = W_qkv[2 * DM : 3 * DM, :][r, :]
    bq = b_qkv[0 * DM + h0 * DH : 0 * DM + (h0 + HPC) * DH]
    bk = b_qkv[1 * DM + h0 * DH : 1 * DM + (h0 + HPC) * DH]
    bv_ = b_qkv[2 * DM + h0 * DH : 2 * DM + (h0 + HPC) * DH]
    return {
        "xT": np.ascontiguousarray(x[g].T.astype(bf16)),
        "wqkT": np.ascontiguousarray(
            np.concatenate([Wq.T, Wk.T], axis=1).astype(bf16)
        ),
        "wvT": np.ascontiguousarray(Wv.T.astype(bf16)),
        "woT": np.ascontiguousarray(W_out[:, r].T.astype(bf16)),
        "bqk": np.concatenate([bq, bk]).reshape(2 * DQ, 1).astype(np.float32),
        "bv": np.ascontiguousarray(
            np.broadcast_to(bv_.reshape(1, DQ), (128, DQ)).astype(np.float32)
        ),
        "tri": tri,
    }


def kernel(x, W_qkv, b_qkv, W_out, b_out, _trace=False):
    x = np.asarray(x)
    W_qkv = np.asarray(W_qkv)
    b_qkv = np.asarray(b_qkv)
    W_out = np.asarray(W_out)
    b_out = np.asarray(b_out)

    # tri[r, c] = (c >= r): keep (k, q) where q >= k in the diagonal block
    tri = np.triu(np.ones((128, 128), dtype=np.float32)).astype(ml_dtypes.bfloat16)

    nc = _get_compiled()
    in_maps = [
        _prep_core_inputs(x, W_qkv, b_qkv, W_out, b_out, c, tri)
        for c in range(N_CORES)
    ]
    res = run_bass_kernel_spmd(nc, in_maps, list(range(N_CORES)), trace=_trace)

    out = np.empty((B, S, DM), dtype=np.float32)
    for g in range(B):
        acc = res.results[g * CPG]["out"].astype(np.float32)
        for lane in range(1, CPG):
            acc = acc + res.results[g * CPG + lane]["out"].astype(np.float32)
        out[g] = acc + b_out[None, :].astype(np.float32)

    if _trace:
        kernel.last_exec_time_ns = res.exec_time_ns
        kernel.last_results = res
    return out
